# revision 2
# baseline (speedup 1.0000x reference)
"""MoE routed conv for Trainium2, 8-core SPMD — 1D Winograd F(2,3),
device-side input transform.

Math: each batch image selects one expert (argmax of scores); output equals
a 3x3 pad-1 conv with the selected expert's filter. We compute only that
expert's conv (5x less work), data-parallel 4 images per core.

v2 changes vs baseline:
- Input DMA halved: host ships raw de-interleaved even/odd padded columns
  (xe/xo, 1.0x input volume) instead of the 2x-inflated Winograd D planes;
  the device computes D0..D3 with 4 contiguous bf16 DVE ops (fast-mode
  eligible: all-SBUF, unit-stride, 16-bit).
- Output stored as parity planes [OC, 2, H, G] (contiguous writes, no
  stride-2 SBUF scatter); host interleaves even/odd columns for free.
- Out-transform engine rebalance: Act drains M1,M2,M3 to SBUF bf16; DVE
  does t01=M0(psum)+a1 (1x) and the two all-SBUF combines (fast mode);
  GpSimd does t12=a1-a2. Each engine stays under the PE's ~7.8us/image.
- Image i+1's input transform is emitted before image i's chunk ops so the
  DVE FIFO never head-blocks; xe/xo tiles are triple-buffered and
  prefetched two images ahead.
"""
import numpy as np

B, C, H, W = 32, 128, 56, 56
E, OC = 5, 128
NCORES = 8
IPC = B // NCORES          # images per core
WE = 29                    # de-interleaved padded width (per parity)
G = W // 2                 # 28 column pairs
CH2 = 14                   # output rows per PSUM chunk
NCH = H // CH2             # 4

_program = None


def _build_program():
    import concourse.bacc as bacc
    import concourse.tile as tile
    from concourse.tile import add_dep_helper
    from concourse import mybir

    dt = mybir.dt
    idt = dt.bfloat16
    nc = bacc.Bacc("TRN2", target_bir_lowering=False, debug=False)
    # x: de-interleaved padded columns, [img, ci, parity(e/o), h, 29]
    x_d = nc.dram_tensor("x", [IPC, C, 2, H, WE], idt, kind="ExternalInput").ap()
    w_d = nc.dram_tensor("w", [IPC, C, 12, OC], idt, kind="ExternalInput").ap()
    # o: parity planes, [img, co, parity(e/o), h, g]
    o_d = nc.dram_tensor("o", [IPC, OC, 2, H, G], idt, kind="ExternalOutput").ap()

    Copy = mybir.ActivationFunctionType.Copy

    with tile.TileContext(nc) as tc:
        with (
            tc.tile_pool(name="xp", bufs=1) as xp,
            tc.tile_pool(name="dp", bufs=1) as dp,
            tc.tile_pool(name="wpool", bufs=1) as wpool,
            tc.tile_pool(name="opool", bufs=1) as opool,
            tc.tile_pool(name="tpool", bufs=24) as tpool,
            tc.tile_pool(name="ps", bufs=8, space="PSUM") as psp,
        ):
            xts = [xp.tile([C, 2, H, WE], idt, name=f"xt{i}") for i in range(3)]
            dts = [dp.tile([C, 4, H, G], idt, name=f"dt{i}") for i in range(2)]
            wts = [wpool.tile([C, 12, OC], idt, name=f"wt{i}") for i in range(IPC)]
            ots = [opool.tile([OC, 2, H, G], idt, name=f"ot{i}") for i in range(2)]

            def load_img(img):
                """input DMAs for one image on the sync queue"""
                xt = xts[img % 3]
                if img == 0:
                    # xe first so D0 (and the first matmuls) can start early
                    l0 = nc.sync.dma_start(out=xt[:, 0:1], in_=x_d[img, :, 0:1])
                    l1 = nc.sync.dma_start(out=wts[img][:], in_=w_d[img])
                    l2 = nc.sync.dma_start(out=xt[:, 1:2], in_=x_d[img, :, 1:2])
                    return [l0, l1, l2]
                a = nc.sync.dma_start(out=xt[:], in_=x_d[img])
                b = nc.sync.dma_start(out=wts[img][:], in_=w_d[img])
                return [a, b]

            def in_transform(img):
                """D0..D3 for one image on DVE; contiguous bf16 SBUF ops."""
                xt = xts[img % 3]
                D = dts[img % 2]
                xe0 = xt[:, 0, :, 0:G]
                xe1 = xt[:, 0, :, 1:G + 1]
                xo0 = xt[:, 1, :, 0:G]
                xo1 = xt[:, 1, :, 1:G + 1]
                nc.vector.tensor_sub(D[:, 0], xe0, xe1)   # d0 - d2
                nc.vector.tensor_add(D[:, 1], xo0, xe1)   # d1 + d2
                nc.vector.tensor_sub(D[:, 2], xe1, xo0)   # d2 - d1
                nc.vector.tensor_sub(D[:, 3], xo0, xo1)   # d1 - d3

            head_loads = load_img(0) + load_img(1)
            in_transform(0)

            anchor = None
            for img in range(IPC):
                dtile = dts[img % 2]
                wt = wts[img]
                ot = ots[img % 2]
                # prefetch two images ahead; delay past the head-critical DMAs
                if img + 2 < IPC:
                    loads = load_img(img + 2)
                    if anchor is not None:
                        for ld in loads:
                            add_dep_helper(ld.ins, anchor.ins, sync=True,
                                           reason="delay prefetch past head")
                # next image's input transform early in this image's DVE window
                if img + 1 < IPC:
                    in_transform(img + 1)

                pss = {}   # (c, j) -> psum tile
                a1 = {}
                a2 = {}
                t01 = {}
                t12 = {}
                # last image runs in two chunk-halves end-to-end so the first
                # half's drains/stores overlap the second half's matmuls
                halves = [(0, 2), (2, 4)] if img == IPC - 1 else [(0, NCH)]
                for (ha, hb) in halves:
                  for j in range(4):
                    for c in range(ha, hb):
                        pss[(c, j)] = psp.tile([OC, CH2, G], dt.float32,
                                               name=f"ps{img}_{c}_{j}", tag="ps")
                    if j == 3 and img == IPC - 1:
                        # final j-group: chunk-outer so each chunk's tail
                        # (a3/oo/store) completes early
                        sweep = [(kh, c) for c in range(ha, hb) for kh in range(3)]
                    else:
                        sweep = [(kh, c) for kh in range(3) for c in range(ha, hb)]
                    for (kh, c) in sweep:
                        r0 = c * CH2
                        hs = max(r0, 1 - kh)
                        he = min(r0 + CH2, H + 1 - kh)
                        rhs = dtile[:, j, hs + kh - 1 : he + kh - 1, :]
                        out = pss[(c, j)][:, hs - r0 : he - r0, :]
                        mm = nc.tensor.matmul(out, wt[:, kh * 4 + j, :], rhs,
                                              start=(kh == 0), stop=(kh == 2))
                        if img == 0 and j == 0 and kh == 0 and c == hb - 1:
                            anchor = mm
                    # output-transform passes that become ready after this j
                    #   oe = M0+M1+M2 = (M0+a1)+a2 ; oo = (a1-a2)-a3
                    if j == 1:
                        for c in range(ha, hb):
                            a = tpool.tile([OC, CH2, G], idt,
                                           name=f"a1_{img}_{c}", tag="tm")
                            a1[c] = a
                            nc.scalar.activation(a[:], pss[(c, 1)][:], Copy)
                            t = tpool.tile([OC, CH2, G], idt,
                                           name=f"t01_{img}_{c}", tag="tm")
                            t01[c] = t
                            nc.vector.tensor_add(t[:], pss[(c, 0)][:], a[:])
                    elif j == 2:
                        for c in range(ha, hb):
                            r0 = c * CH2
                            a = tpool.tile([OC, CH2, G], idt,
                                           name=f"a2_{img}_{c}", tag="tm")
                            a2[c] = a
                            nc.scalar.activation(a[:], pss[(c, 2)][:], Copy)
                            t = tpool.tile([OC, CH2, G], idt,
                                           name=f"t12_{img}_{c}", tag="tm")
                            t12[c] = t
                            nc.gpsimd.tensor_sub(t[:], a1[c][:], a[:])
                            nc.vector.tensor_add(ot[:, 0, r0:r0 + CH2, :],
                                                 t01[c][:], a[:])
                    elif j == 3:
                        for c in range(ha, hb):
                            r0 = c * CH2
                            a = tpool.tile([OC, CH2, G], idt,
                                           name=f"a3_{img}_{c}", tag="tm")
                            nc.scalar.activation(a[:], pss[(c, 3)][:], Copy)
                            nc.vector.tensor_sub(ot[:, 1, r0:r0 + CH2, :],
                                                 t12[c][:], a[:])
                            if img == IPC - 1:
                                # per-half stores; final store on the gpsimd
                                # queue (cheap DGE dispatch, idle engine)
                                if c == 1:
                                    nc.sync.dma_start(
                                        out=o_d[img, :, :, 0:28, :],
                                        in_=ot[:, :, 0:28, :])
                                elif c == 3:
                                    nc.gpsimd.dma_start(
                                        out=o_d[img, :, :, 28:56, :],
                                        in_=ot[:, :, 28:56, :])
                        if img < IPC - 1:
                            nc.gpsimd.dma_start(out=o_d[img], in_=ot[:])
    nc.compile()
    return nc


def _get_program():
    global _program
    if _program is None:
        _program = _build_program()
    return _program


def kernel(x: np.ndarray, scores: np.ndarray, weight: np.ndarray,
           **run_kwargs) -> np.ndarray:
    import ml_dtypes
    from concourse.bass_utils import run_bass_kernel_spmd

    x = np.asarray(x, dtype=np.float32)
    scores = np.asarray(scores, dtype=np.float32)
    weight = np.asarray(weight, dtype=np.float32)

    expert = np.argmax(scores, axis=1)                       # [B]
    w_sel = weight.reshape(E, OC, C, 3, 3)[expert]           # [B, co, ci, kh, kw]
    # Winograd weight transform G.w per kh: [B, co, ci, kh, j]
    w0, w1, w2 = w_sel[..., 0], w_sel[..., 1], w_sel[..., 2]
    wt = np.stack([w0, (w0 + w1 + w2) * 0.5, (w0 - w1 + w2) * 0.5, w2], axis=-1)
    # lhsT layout: [ci, kh*4+j, co]
    w_lhsT = np.ascontiguousarray(
        wt.transpose(0, 2, 3, 4, 1).reshape(B, C, 12, OC)).astype(ml_dtypes.bfloat16)

    # de-interleaved padded columns: xe[k]=xpad[2k], xo[k]=xpad[2k+1]
    xeo = np.zeros((B, C, 2, H, WE), np.float32)
    xeo[:, :, 0, :, 1:WE] = x[:, :, :, 1::2]    # xe: cols 1,3..55
    xeo[:, :, 1, :, 0:G] = x[:, :, :, 0::2]     # xo: cols 0,2..54
    xeo = xeo.astype(ml_dtypes.bfloat16)

    nc = _get_program()
    in_maps = [
        {"x": xeo[k * IPC : (k + 1) * IPC], "w": w_lhsT[k * IPC : (k + 1) * IPC]}
        for k in range(NCORES)
    ]
    res = run_bass_kernel_spmd(nc, in_maps, list(range(NCORES)), **run_kwargs)
    o = np.concatenate([res.results[k]["o"] for k in range(NCORES)], axis=0)
    o = o.astype(np.float32)                     # [B, OC, 2, H, G]
    out = np.empty((B, OC, H, W), np.float32)
    out[:, :, :, 0::2] = o[:, :, 0]
    out[:, :, :, 1::2] = o[:, :, 1]
    if run_kwargs:
        kernel.last_results = res
    return out
